# revision 52
# baseline (speedup 1.0000x reference)
"""Trainium2 Bass kernel for DiagonalSSMLayer.

Math: y = C_w @ h + D*u  where  h[l] = lam*h[l-1] + (B_w @ u)[l]  (per state
channel, lam = sigmoid(log_lambda)).  The reference computes the causal
exponential-decay convolution via FFT; here it is the exact linear recurrence,
done with the native tensor_tensor_scan (fp32 internal state).

Sharding: 8 cores = (batch b in 0..3) x (sequence half s in 0..1).
Each core gets u[b, s*2048:(s+1)*2048, :] transposed so the contraction dim d
sits on SBUF partitions for both GEMMs.  All GEMM operands are bf16 (full PE
rate, half the HBM traffic, 4x cheaper LDWEIGHTS); PSUM accumulation and the
scan state stay fp32; h and y are written bf16.  Rel err ~5e-3 (gate 2e-2).

Cross-half carry: second-half cores prepend a HALO of the last `HALO`
positions of the first half and scan through it, reconstructing the incoming
state up to lam^HALO (~3e-3) -- below bf16 rounding noise.  First-half cores
get a zero halo (uniform SPMD program).

DMA: TRN2 exposes two hardware DGE queues (SP + ACT) plus gpsimd's software
DGE (runs on the Pool cores).  Measured on this part (micro-benchmarked):
throughput depends almost entirely on the per-partition contiguous run
length -- 2KB runs ~90 GB/s, 8KB ~235, 32KB ~375 -- and a second queue adds
little (shared backend).  Everything is therefore laid out flat
chunk-contiguous per partition: u chunks move as single 1MB transfers with
8KB runs, y stores accumulate a full chunk in SBUF [128, KT*LC] and leave
as one 8KB-run transfer, weights are 4KB runs.  A queue round-robins among
outstanding transfers (issue order gives no priority), so the u loads are
CHAINED in need-order, each gated on the previous completing via a
single-packet dependency-carrying dummy DMA.  The small params ride the
software DGE (128 tiny packets each would waste hardware dispatch).

Engine split per 512-col chunk (PE ~7us of matmuls is the bound): DVE runs
both scans from PSUM + even-k y-fuse; ACT drains odd-k C@h from PSUM to SBUF
bf16 and premultiplies D*u; Pool adds the two (all-SBUF).  Pool cannot touch
PSUM, scan/fused-stt lower only on DVE, and Pool's tensor_scalar is a slow
software loop -- hence this exact split.
"""

import numpy as np

B, L, DM, NS = 4, 4096, 1024, 256
HALF = L // 2          # 2048 sequence positions per core
NCORES = 8
LC = 512               # l-chunk (matmul free dim / scan chunk)
NLC = HALF // LC       # 4 main chunks
HALO = 192
KT = DM // 128         # 8 k-tiles (contraction over d)
NT = NS // 128         # 2 n-tiles (state channels)

HKW = 4 * HALO         # per-region halo width (4 k-tiles)
BKW = 4 * NS           # per-region B_w width
CKW = 4 * LC           # per-region u-chunk width
RW = HKW + BKW + NLC * CKW   # SBUF region width (one of two k-groups)
# per-region DRAM layout: [uh | dup | Bw | dup | c0 | dup | c1 | dup | c2 |
# dup | c3]; each dup column repeats the next transfer's first column so
# consecutive transfers overlap-write one SBUF column (true WAW chaining)
UALL_W = RW + NLC + 1

_CACHE = {}


def _build(warm=4):
    from concourse import bacc, tile, mybir

    MULT = mybir.AluOpType.mult
    ADD = mybir.AluOpType.add
    f32 = mybir.dt.float32
    bf16 = mybir.dt.bfloat16

    nc = bacc.Bacc("TRN2", target_bir_lowering=False, debug=False,
                   num_devices=NCORES)

    # flat, chunk-contiguous layouts (big DMA packets); two regions back to
    # back (k-tiles 0-3, then 4-7)
    uall_d = nc.dram_tensor("uall", [128, 2 * UALL_W], bf16, kind="ExternalInput").ap()
    CwT_d = nc.dram_tensor("CwT", [128, NT * DM], bf16, kind="ExternalInput").ap()
    lam_d = nc.dram_tensor("lamvec", [NS, 1], f32, kind="ExternalInput").ap()
    dvec_d = nc.dram_tensor("dvec", [128, KT], f32, kind="ExternalInput").ap()
    yT_d = nc.dram_tensor("yT", [128, NLC * KT * LC], bf16, kind="ExternalOutput").ap()

    with tile.TileContext(nc) as tc:
        with tc.tile_pool(name="const", bufs=1) as cpool, \
             tc.tile_pool(name="u", bufs=1) as upool, \
             tc.tile_pool(name="h", bufs=1) as hpool, \
             tc.tile_pool(name="y", bufs=4) as ypool, \
             tc.tile_pool(name="bu_ps", bufs=3, space="PSUM") as bupool, \
             tc.tile_pool(name="y_ps", bufs=5, space="PSUM") as yppool:

            # ---- warmup constant + tiny params on the Pool queue
            warm_sb = cpool.tile([128, 512], bf16, name="warm")
            nc.gpsimd.memset(warm_sb[:], 1.0)
            lamv_sb = [cpool.tile([128, 1], f32, name=f"lamv{n}") for n in range(NT)]
            for n in range(NT):
                nc.gpsimd.dma_start(out=lamv_sb[n][:], in_=lam_d[n * 128:(n + 1) * 128, :])
            dvec3 = cpool.tile([128, KT], f32, name="dv")
            nc.gpsimd.dma_start(out=dvec3[:], in_=dvec_d[:, :])
            dvec_sb = [dvec3[:, k:k + 1] for k in range(KT)]



            # ---- bulk loads: TWO independent chained streams on the SP
            # queue (k-tiles 0-3 in region A, 4-7 in region B), so two
            # transfers are always outstanding (the DMA backend needs ~2
            # concurrent transfers for full rate) while staying strictly in
            # need-order within each stream.  Each transfer's destination
            # overlaps the next transfer's first SBUF column (same value,
            # duplicated host-side) -- a true WAW dependency the scheduler
            # cannot reorder away.
            ubig = [upool.tile([128, RW], bf16, name=f"ubig{g}")
                    for g in range(2)]
            uh_sb = [ubig[k // 4][:, (k % 4) * HALO:(k % 4 + 1) * HALO]
                     for k in range(KT)]
            BwT_sb = [ubig[k // 4][:, HKW + (k % 4) * NS:HKW + (k % 4 + 1) * NS]
                      for k in range(KT)]

            def uC(c, k):
                base = HKW + BKW + c * CKW + (k % 4) * LC
                return ubig[k // 4][:, base:base + LC]

            # per-region transfers: (dst, src, width); dups shift src by one
            # per prior transfer
            tr = [(0, 0, HKW + 1), (HKW, HKW + 1, BKW + 1)]
            for c in range(NLC):
                w = CKW + 1 if c < NLC - 1 else CKW
                tr.append((HKW + BKW + c * CKW, HKW + BKW + 2 + c * (CKW + 1), w))
            for dst, srcs, w in tr:
                for g in range(2):
                    s0 = g * UALL_W + srcs
                    nc.sync.dma_start(out=ubig[g][:, dst:dst + w],
                                      in_=uall_d[:, s0:s0 + w])

            # CwT rides the software DGE (separate backend), gated past the
            # startup crunch by a true data dependency on the Bw transfer
            scr2 = cpool.tile([128, 1], bf16, name="scr2")
            gc = HKW + BKW - 1  # last B_w column: written only by the Bw transfer
            nc.gpsimd.tensor_tensor(scr2[:], ubig[0][:, gc:gc + 1],
                                    ubig[1][:, gc:gc + 1], ADD)
            CwT3 = cpool.tile([128, NT * DM], bf16, name="cw")
            nc.gpsimd.dma_start(out=CwT3[:], in_=CwT_d[:, :])
            CwT_sb = [CwT3[:, n * DM:(n + 1) * DM] for n in range(NT)]

            # lam broadcast tiles (scans run on DVE; build them there too)
            lam_sb = [cpool.tile([128, LC], f32, name=f"lam{n}") for n in range(NT)]
            for n in range(NT):
                nc.vector.memset(lam_sb[n][:], 1.0)
                nc.vector.tensor_scalar_mul(lam_sb[n][:], lam_sb[n][:], lamv_sb[n][:])

            # ---- PE warmup: dummy matmuls nudge the clock ramp while the
            # halo+Bw DMA streams
            if warm:
                warm_ps = yppool.tile([128, LC], f32, tag="y")
                for w in range(warm):
                    nc.tensor.matmul(warm_ps[:], warm_sb[:, 0:128], warm_sb[:],
                                     start=(w == 0), stop=(w == warm - 1))

            hr = [hpool.tile([128, HALF], bf16, name=f"hr_{n}") for n in range(NT)]
            hh = [hpool.tile([128, HALO], bf16, name=f"hh{n}") for n in range(NT)]

            # ---- halo: GEMM1 + scan over the carry-reconstruction region
            for n in range(NT):
                bu_ps = bupool.tile([128, LC], f32, tag="bu")
                for k in range(KT):
                    nc.tensor.matmul(bu_ps[:, 0:HALO],
                                     BwT_sb[k][:, n * 128:(n + 1) * 128],
                                     uh_sb[k],
                                     start=(k == 0), stop=(k == KT - 1))
                nc.vector.tensor_tensor_scan(
                    hh[n][:], lam_sb[n][:, 0:HALO], bu_ps[:, 0:HALO],
                    0.0, MULT, ADD)

            # second warm burst keeps the PE clock ramp alive across the
            # short gap between the halo GEMM and chunk0's data arriving
            warm_ps2 = yppool.tile([128, LC], f32, tag="y")
            for w in range(4):
                nc.tensor.matmul(warm_ps2[:], warm_sb[:, 0:128], warm_sb[:],
                                 start=(w == 0), stop=(w == 3))

            # ---- main chunks: GEMM1 -> scan -> GEMM2 -> y out.
            # GEMM2/y ops are software-pipelined one chunk behind the scan
            # chain so the next scan never queues behind the previous chunk's
            # y ops on the in-order DVE.
            def gemm2(c):
                o = c * LC
                y8_sb = ypool.tile([128, KT, LC], bf16, tag="ysb")
                for k in range(KT):
                    y_ps = yppool.tile([128, LC], f32, tag="y")
                    for n in range(NT):
                        nc.tensor.matmul(y_ps[:],
                                         CwT_sb[n][:, k * 128:(k + 1) * 128],
                                         hr[n][:, o:o + LC],
                                         start=(n == 0), stop=(n == NT - 1))
                    if k % 2 == 0:
                        nc.vector.scalar_tensor_tensor(
                            y8_sb[:, k, :], uC(c, k),
                            dvec_sb[k], y_ps[:], MULT, ADD)
                    else:
                        ch_sb = ypool.tile([128, LC], bf16, tag="chsb")
                        nc.scalar.copy(ch_sb[:], y_ps[:])
                        ud_sb = ypool.tile([128, LC], bf16, tag="udsb")
                        nc.scalar.mul(ud_sb[:], uC(c, k), dvec_sb[k])
                        nc.gpsimd.tensor_tensor(
                            y8_sb[:, k, :], ud_sb[:], ch_sb[:], ADD)
                # y stores all ride the ACT queue: stores use the DMA write
                # path, which runs concurrently with the SP queue's loads
                base = c * KT * LC
                if c == NLC - 1:
                    # split the last store so draining starts mid-gemm2
                    half = KT // 2 * LC
                    nc.scalar.dma_start(out=yT_d[:, base:base + half],
                                        in_=y8_sb[:, 0:KT // 2, :])
                    nc.scalar.dma_start(out=yT_d[:, base + half:base + KT * LC],
                                        in_=y8_sb[:, KT // 2:KT, :])
                else:
                    nc.scalar.dma_start(out=yT_d[:, base:base + KT * LC],
                                        in_=y8_sb[:])

            for c in range(NLC):
                o = c * LC
                for n in range(NT):
                    bu_ps = bupool.tile([128, LC], f32, tag="bu")
                    for k in range(KT):
                        nc.tensor.matmul(bu_ps[:],
                                         BwT_sb[k][:, n * 128:(n + 1) * 128],
                                         uC(c, k),
                                         start=(k == 0), stop=(k == KT - 1))
                    init = (hh[n][:, HALO - 1:HALO] if c == 0
                            else hr[n][:, o - 1:o])
                    nc.vector.tensor_tensor_scan(
                        hr[n][:, o:o + LC],
                        lam_sb[n][:], bu_ps[:], init, MULT, ADD)
                if c > 0:
                    gemm2(c - 1)
            gemm2(NLC - 1)

    nc.compile()
    return nc


def _sigmoid(x):
    return 1.0 / (1.0 + np.exp(-x))


def kernel(u, log_lambda, B_w, C_w, D):
    import ml_dtypes
    from concourse.bass_utils import run_bass_kernel_spmd

    bf16 = ml_dtypes.bfloat16

    if "nc" not in _CACHE:
        _CACHE["nc"] = _build()
    nc = _CACHE["nc"]

    lam = _sigmoid(np.asarray(log_lambda, dtype=np.float64))
    # [128, KT*N] flat k-major layouts: row p of k-block k holds d = k*128+p
    BwT = np.ascontiguousarray(
        np.asarray(B_w, np.float32).T.reshape(KT, 128, NS)
        .transpose(1, 0, 2).reshape(128, KT * NS)).astype(bf16)
    CwT = np.ascontiguousarray(
        np.asarray(C_w, np.float32).T.reshape(NT, 128, DM)
        .transpose(1, 0, 2).reshape(128, NT * DM)).astype(bf16)
    dvec = np.ascontiguousarray(np.asarray(D, np.float32).reshape(KT, 128).T)
    lamvec = np.ascontiguousarray(lam.reshape(NS, 1)).astype(np.float32)

    ub = np.asarray(u, dtype=np.float32).astype(bf16)

    def flat_cols(blk):  # [cols, DM] -> [128, KT*cols], chunk-contiguous
        cols = blk.shape[0]
        return (blk.T.reshape(KT, 128, cols).transpose(1, 0, 2)
                .reshape(128, KT * cols))

    in_maps = []
    for core in range(NCORES):
        b, s = core // 2, core % 2
        lo = s * HALF
        uhv = (flat_cols(ub[b, lo - HALO:lo, :]) if s == 1
               else np.zeros((128, KT * HALO), dtype=bf16))
        chunks = [flat_cols(ub[b, lo + c * LC:lo + (c + 1) * LC, :])
                  for c in range(NLC)]
        uall = np.zeros((128, 2 * UALL_W), dtype=bf16)
        for g in range(2):
            o = g * UALL_W
            ks = slice(g * HKW, (g + 1) * HKW)      # this region's halo cols
            uall[:, o:o + HKW] = uhv[:, ks]
            bwg = BwT[:, g * BKW:(g + 1) * BKW]
            uall[:, o + HKW] = bwg[:, 0]            # dup: Bw first col
            uall[:, o + HKW + 1:o + HKW + 1 + BKW] = bwg
            cg = [ch[:, g * CKW:(g + 1) * CKW] for ch in chunks]
            uall[:, o + HKW + 1 + BKW] = cg[0][:, 0]  # dup: c0 first col
            for c in range(NLC):
                p = o + HKW + BKW + 2 + c * (CKW + 1)
                uall[:, p:p + CKW] = cg[c]
                if c < NLC - 1:
                    uall[:, p + CKW] = cg[c + 1][:, 0]  # dup boundary col
        in_maps.append({
            "uall": uall,
            "CwT": CwT,
            "lamvec": lamvec,
            "dvec": dvec,
        })
    _CACHE["in_maps"] = in_maps

    def _run():
        return run_bass_kernel_spmd(nc, in_maps, core_ids=list(range(NCORES)))

    try:
        res = _run()
    except Exception:
        # a previously failed execution can wedge the backend; reset + retry
        try:
            import ctypes, jax
            jax.devices()
            lib = ctypes.CDLL("/opt/axon/libaxon_pjrt.so")
            lib.axon_reset.restype = ctypes.c_int64
            lib.axon_reset()
        except Exception:
            pass
        res = _run()

    y = np.empty((B, L, DM), dtype=np.float32)
    for core in range(NCORES):
        b, s = core // 2, core % 2
        # yT flat [128, c, k, l] -> y[b, lo + c*LC + l, k*128 + p]
        yt = res.results[core]["yT"].reshape(128, NLC, KT, LC).astype(np.float32)
        y[b, s * HALF:(s + 1) * HALF, :] = (
            yt.transpose(1, 3, 2, 0).reshape(HALF, DM))
    return y


# revision 55
# speedup vs baseline: 1.0539x; 1.0539x over previous
"""Trainium2 Bass kernel for DiagonalSSMLayer.

Math: y = C_w @ h + D*u  where  h[l] = lam*h[l-1] + (B_w @ u)[l]  (per state
channel, lam = sigmoid(log_lambda)).  The reference computes the causal
exponential-decay convolution via FFT; here it is the exact linear recurrence,
done with the native tensor_tensor_scan (fp32 internal state).

Sharding: 8 cores = (batch b in 0..3) x (sequence half s in 0..1).
Each core gets u[b, s*2048:(s+1)*2048, :] transposed so the contraction dim d
sits on SBUF partitions for both GEMMs.  All GEMM operands are bf16 (full PE
rate, half the HBM traffic, 4x cheaper LDWEIGHTS); PSUM accumulation and the
scan state stay fp32; h and y are written bf16.  Rel err ~5e-3 (gate 2e-2).

Cross-half carry: second-half cores prepend a HALO of the last `HALO`
positions of the first half and scan through it, reconstructing the incoming
state up to lam^HALO (~3e-3) -- below bf16 rounding noise.  First-half cores
get a zero halo (uniform SPMD program).

DMA: TRN2 exposes two hardware DGE queues (SP + ACT) plus gpsimd's software
DGE (runs on the Pool cores).  Measured on this part (micro-benchmarked):
throughput depends almost entirely on the per-partition contiguous run
length -- 2KB runs ~90 GB/s, 8KB ~235, 32KB ~375 -- and a second queue adds
little (shared backend).  Everything is therefore laid out flat
chunk-contiguous per partition: u chunks move as single 1MB transfers with
8KB runs, y stores accumulate a full chunk in SBUF [128, KT*LC] and leave
as one 8KB-run transfer, weights are 4KB runs.  A queue round-robins among
outstanding transfers (issue order gives no priority), so the u loads are
CHAINED in need-order, each gated on the previous completing via a
single-packet dependency-carrying dummy DMA.  The small params ride the
software DGE (128 tiny packets each would waste hardware dispatch).

Engine split per 512-col chunk (PE ~7us of matmuls is the bound): DVE runs
both scans from PSUM + even-k y-fuse; ACT drains odd-k C@h from PSUM to SBUF
bf16 and premultiplies D*u; Pool adds the two (all-SBUF).  Pool cannot touch
PSUM, scan/fused-stt lower only on DVE, and Pool's tensor_scalar is a slow
software loop -- hence this exact split.
"""

import numpy as np

B, L, DM, NS = 4, 4096, 1024, 256
HALF = L // 2          # 2048 sequence positions per core
NCORES = 8
LC = 512               # l-chunk (matmul free dim / scan chunk)
NLC = HALF // LC       # 4 main chunks
HALO = 192
KT = DM // 128         # 8 k-tiles (contraction over d)
NT = NS // 128         # 2 n-tiles (state channels)

HKW = 4 * HALO         # per-region halo width (4 k-tiles)
BKW = 4 * NS           # per-region B_w width
CKW = 4 * LC           # per-region u-chunk width
RW = HKW + BKW + NLC * CKW   # SBUF region width (one of two k-groups)
# per-region DRAM layout: [uh | dup | Bw | dup | c0 | dup | c1 | dup | c2 |
# dup | c3]; each dup column repeats the next transfer's first column so
# consecutive transfers overlap-write one SBUF column (true WAW chaining)
UALL_W = RW + NLC + 1

_CACHE = {}


def _build(warm=4):
    from concourse import bacc, tile, mybir

    MULT = mybir.AluOpType.mult
    ADD = mybir.AluOpType.add
    f32 = mybir.dt.float32
    bf16 = mybir.dt.bfloat16

    nc = bacc.Bacc("TRN2", target_bir_lowering=False, debug=False,
                   num_devices=NCORES)

    # flat, chunk-contiguous layouts (big DMA packets); two regions back to
    # back (k-tiles 0-3, then 4-7)
    uall_d = nc.dram_tensor("uall", [128, 2 * UALL_W], bf16, kind="ExternalInput").ap()
    CwT_d = nc.dram_tensor("CwT", [128, NT * DM], bf16, kind="ExternalInput").ap()
    lam_d = nc.dram_tensor("lamvec", [NS, 1], f32, kind="ExternalInput").ap()
    dvec_d = nc.dram_tensor("dvec", [128, KT], f32, kind="ExternalInput").ap()
    yT_d = nc.dram_tensor("yT", [128, NLC * KT * LC], bf16, kind="ExternalOutput").ap()

    with tile.TileContext(nc) as tc:
        with tc.tile_pool(name="const", bufs=1) as cpool, \
             tc.tile_pool(name="u", bufs=1) as upool, \
             tc.tile_pool(name="h", bufs=1) as hpool, \
             tc.tile_pool(name="y", bufs=4) as ypool, \
             tc.tile_pool(name="bu_ps", bufs=3, space="PSUM") as bupool, \
             tc.tile_pool(name="y_ps", bufs=5, space="PSUM") as yppool:

            # ---- warmup constant + tiny params on the Pool queue
            warm_sb = cpool.tile([128, 512], bf16, name="warm")
            nc.gpsimd.memset(warm_sb[:], 1.0)
            lamv_sb = [cpool.tile([128, 1], f32, name=f"lamv{n}") for n in range(NT)]
            for n in range(NT):
                nc.gpsimd.dma_start(out=lamv_sb[n][:], in_=lam_d[n * 128:(n + 1) * 128, :])
            dvec3 = cpool.tile([128, KT], f32, name="dv")
            nc.gpsimd.dma_start(out=dvec3[:], in_=dvec_d[:, :])
            dvec_sb = [dvec3[:, k:k + 1] for k in range(KT)]



            # ---- bulk loads: TWO independent chained streams on the SP
            # queue (k-tiles 0-3 in region A, 4-7 in region B), so two
            # transfers are always outstanding (the DMA backend needs ~2
            # concurrent transfers for full rate) while staying strictly in
            # need-order within each stream.  Each transfer's destination
            # overlaps the next transfer's first SBUF column (same value,
            # duplicated host-side) -- a true WAW dependency the scheduler
            # cannot reorder away.
            ubig = [upool.tile([128, RW], bf16, name=f"ubig{g}")
                    for g in range(2)]
            uh_sb = [ubig[k // 4][:, (k % 4) * HALO:(k % 4 + 1) * HALO]
                     for k in range(KT)]
            BwT_sb = [ubig[k // 4][:, HKW + (k % 4) * NS:HKW + (k % 4 + 1) * NS]
                      for k in range(KT)]

            def uC(c, k):
                base = HKW + BKW + c * CKW + (k % 4) * LC
                return ubig[k // 4][:, base:base + LC]

            # per-region transfers: (dst, src, width); dups shift src by one
            # per prior transfer
            tr = [(0, 0, HKW + 1), (HKW, HKW + 1, BKW + 1)]
            for c in range(NLC):
                w = CKW + 1 if c < NLC - 1 else CKW
                tr.append((HKW + BKW + c * CKW, HKW + BKW + 2 + c * (CKW + 1), w))
            for dst, srcs, w in tr:
                for g in range(2):
                    s0 = g * UALL_W + srcs
                    nc.sync.dma_start(out=ubig[g][:, dst:dst + w],
                                      in_=uall_d[:, s0:s0 + w])

            # CwT rides the software DGE (separate backend), gated past the
            # startup crunch by a true data dependency on the Bw transfer
            scr2 = cpool.tile([128, 1], bf16, name="scr2")
            gc = HKW + BKW - 1  # last B_w column: written only by the Bw transfer
            nc.gpsimd.tensor_tensor(scr2[:], ubig[0][:, gc:gc + 1],
                                    ubig[1][:, gc:gc + 1], ADD)
            CwT3 = cpool.tile([128, NT * DM], bf16, name="cw")
            nc.gpsimd.dma_start(out=CwT3[:], in_=CwT_d[:, :])
            CwT_sb = [CwT3[:, n * DM:(n + 1) * DM] for n in range(NT)]

            # lam broadcast tiles (scans run on DVE; build them there too)
            lam_sb = [cpool.tile([128, LC], f32, name=f"lam{n}") for n in range(NT)]
            for n in range(NT):
                nc.vector.memset(lam_sb[n][:], 1.0)
                nc.vector.tensor_scalar_mul(lam_sb[n][:], lam_sb[n][:], lamv_sb[n][:])

            # ---- PE warmup: dummy matmuls keep the PE clock ramp alive
            # while the loads stream.  Burst 1 has no deps (starts at queue
            # drain); burst 2 reads the halo tile so it slots in right after
            # the uh transfers land, bridging to the halo GEMM (which also
            # needs B_w).
            warm_ps = yppool.tile([128, LC], f32, tag="y")
            for w in range(7):
                nc.tensor.matmul(warm_ps[:], warm_sb[:, 0:128], warm_sb[:],
                                 start=(w == 0), stop=(w == 6))
            warm_ps2 = yppool.tile([128, LC], f32, tag="y")
            for w in range(5):
                nc.tensor.matmul(warm_ps2[:], uh_sb[0][:, 0:128], warm_sb[:],
                                 start=(w == 0), stop=(w == 4))

            hr = [hpool.tile([128, HALF], bf16, name=f"hr_{n}") for n in range(NT)]
            hh = [hpool.tile([128, HALO], bf16, name=f"hh{n}") for n in range(NT)]

            # ---- halo: GEMM1 + scan over the carry-reconstruction region
            for n in range(NT):
                bu_ps = bupool.tile([128, LC], f32, tag="bu")
                for k in range(KT):
                    nc.tensor.matmul(bu_ps[:, 0:HALO],
                                     BwT_sb[k][:, n * 128:(n + 1) * 128],
                                     uh_sb[k],
                                     start=(k == 0), stop=(k == KT - 1))
                nc.vector.tensor_tensor_scan(
                    hh[n][:], lam_sb[n][:, 0:HALO], bu_ps[:, 0:HALO],
                    0.0, MULT, ADD)

            # burst 3 bridges the gap between the halo GEMM and chunk0
            warm_ps3 = yppool.tile([128, LC], f32, tag="y")
            for w in range(8):
                nc.tensor.matmul(warm_ps3[:], uh_sb[1][:, 0:128], warm_sb[:],
                                 start=(w == 0), stop=(w == 7))

            # ---- main chunks: GEMM1 -> scan -> GEMM2 -> y out.
            # GEMM2/y ops are software-pipelined one chunk behind the scan
            # chain so the next scan never queues behind the previous chunk's
            # y ops on the in-order DVE.
            def gemm2(c):
                o = c * LC
                y8_sb = ypool.tile([128, KT, LC], bf16, tag="ysb")
                for k in range(KT):
                    y_ps = yppool.tile([128, LC], f32, tag="y")
                    for n in range(NT):
                        nc.tensor.matmul(y_ps[:],
                                         CwT_sb[n][:, k * 128:(k + 1) * 128],
                                         hr[n][:, o:o + LC],
                                         start=(n == 0), stop=(n == NT - 1))
                    # split the y-fuse DVE/ACT+Pool; the last chunk runs all-
                    # DVE (the ACT->Pool chain has ~2.5us latency -- fine
                    # mid-kernel, too slow for the drain tail)
                    if k % 2 == 0 or c == NLC - 1:
                        nc.vector.scalar_tensor_tensor(
                            y8_sb[:, k, :], uC(c, k),
                            dvec_sb[k], y_ps[:], MULT, ADD)
                    else:
                        ch_sb = ypool.tile([128, LC], bf16, tag="chsb")
                        nc.scalar.copy(ch_sb[:], y_ps[:])
                        ud_sb = ypool.tile([128, LC], bf16, tag="udsb")
                        nc.scalar.mul(ud_sb[:], uC(c, k), dvec_sb[k])
                        nc.gpsimd.tensor_tensor(
                            y8_sb[:, k, :], ud_sb[:], ch_sb[:], ADD)
                # y stores all ride the ACT queue: stores use the DMA write
                # path, which runs concurrently with the SP queue's loads
                base = c * KT * LC
                if c == NLC - 1:
                    # split the last store so draining starts mid-gemm2
                    half = KT // 2 * LC
                    nc.scalar.dma_start(out=yT_d[:, base:base + half],
                                        in_=y8_sb[:, 0:KT // 2, :])
                    nc.scalar.dma_start(out=yT_d[:, base + half:base + KT * LC],
                                        in_=y8_sb[:, KT // 2:KT, :])
                else:
                    nc.scalar.dma_start(out=yT_d[:, base:base + KT * LC],
                                        in_=y8_sb[:])

            for c in range(NLC):
                o = c * LC
                for n in range(NT):
                    bu_ps = bupool.tile([128, LC], f32, tag="bu")
                    for k in range(KT):
                        nc.tensor.matmul(bu_ps[:],
                                         BwT_sb[k][:, n * 128:(n + 1) * 128],
                                         uC(c, k),
                                         start=(k == 0), stop=(k == KT - 1))
                    init = (hh[n][:, HALO - 1:HALO] if c == 0
                            else hr[n][:, o - 1:o])
                    nc.vector.tensor_tensor_scan(
                        hr[n][:, o:o + LC],
                        lam_sb[n][:], bu_ps[:], init, MULT, ADD)
                if c > 0:
                    gemm2(c - 1)
            gemm2(NLC - 1)

    nc.compile()
    return nc


def _sigmoid(x):
    return 1.0 / (1.0 + np.exp(-x))


def kernel(u, log_lambda, B_w, C_w, D):
    import ml_dtypes
    from concourse.bass_utils import run_bass_kernel_spmd

    bf16 = ml_dtypes.bfloat16

    if "nc" not in _CACHE:
        _CACHE["nc"] = _build()
    nc = _CACHE["nc"]

    lam = _sigmoid(np.asarray(log_lambda, dtype=np.float64))
    # [128, KT*N] flat k-major layouts: row p of k-block k holds d = k*128+p
    BwT = np.ascontiguousarray(
        np.asarray(B_w, np.float32).T.reshape(KT, 128, NS)
        .transpose(1, 0, 2).reshape(128, KT * NS)).astype(bf16)
    CwT = np.ascontiguousarray(
        np.asarray(C_w, np.float32).T.reshape(NT, 128, DM)
        .transpose(1, 0, 2).reshape(128, NT * DM)).astype(bf16)
    dvec = np.ascontiguousarray(np.asarray(D, np.float32).reshape(KT, 128).T)
    lamvec = np.ascontiguousarray(lam.reshape(NS, 1)).astype(np.float32)

    ub = np.asarray(u, dtype=np.float32).astype(bf16)

    def flat_cols(blk):  # [cols, DM] -> [128, KT*cols], chunk-contiguous
        cols = blk.shape[0]
        return (blk.T.reshape(KT, 128, cols).transpose(1, 0, 2)
                .reshape(128, KT * cols))

    in_maps = []
    for core in range(NCORES):
        b, s = core // 2, core % 2
        lo = s * HALF
        uhv = (flat_cols(ub[b, lo - HALO:lo, :]) if s == 1
               else np.zeros((128, KT * HALO), dtype=bf16))
        chunks = [flat_cols(ub[b, lo + c * LC:lo + (c + 1) * LC, :])
                  for c in range(NLC)]
        uall = np.zeros((128, 2 * UALL_W), dtype=bf16)
        for g in range(2):
            o = g * UALL_W
            ks = slice(g * HKW, (g + 1) * HKW)      # this region's halo cols
            uall[:, o:o + HKW] = uhv[:, ks]
            bwg = BwT[:, g * BKW:(g + 1) * BKW]
            uall[:, o + HKW] = bwg[:, 0]            # dup: Bw first col
            uall[:, o + HKW + 1:o + HKW + 1 + BKW] = bwg
            cg = [ch[:, g * CKW:(g + 1) * CKW] for ch in chunks]
            uall[:, o + HKW + 1 + BKW] = cg[0][:, 0]  # dup: c0 first col
            for c in range(NLC):
                p = o + HKW + BKW + 2 + c * (CKW + 1)
                uall[:, p:p + CKW] = cg[c]
                if c < NLC - 1:
                    uall[:, p + CKW] = cg[c + 1][:, 0]  # dup boundary col
        in_maps.append({
            "uall": uall,
            "CwT": CwT,
            "lamvec": lamvec,
            "dvec": dvec,
        })
    _CACHE["in_maps"] = in_maps

    def _run():
        return run_bass_kernel_spmd(nc, in_maps, core_ids=list(range(NCORES)))

    try:
        res = _run()
    except Exception:
        # a previously failed execution can wedge the backend; reset + retry
        try:
            import ctypes, jax
            jax.devices()
            lib = ctypes.CDLL("/opt/axon/libaxon_pjrt.so")
            lib.axon_reset.restype = ctypes.c_int64
            lib.axon_reset()
        except Exception:
            pass
        res = _run()

    y = np.empty((B, L, DM), dtype=np.float32)
    for core in range(NCORES):
        b, s = core // 2, core % 2
        # yT flat [128, c, k, l] -> y[b, lo + c*LC + l, k*128 + p]
        yt = res.results[core]["yT"].reshape(128, NLC, KT, LC).astype(np.float32)
        y[b, s * HALF:(s + 1) * HALF, :] = (
            yt.transpose(1, 3, 2, 0).reshape(HALF, DM))
    return y
